# revision 13
# baseline (speedup 1.0000x reference)
"""Fused multi-head causal attention (RoPE) for Trainium2, 8-core SPMD.

Sharding: data-parallel over batch (B=2) x tensor-parallel over heads
(16 heads -> 4 per core, Megatron-style column/row split of the qkv/out
projections). Each core computes a partial (T, C) output; the host sums
the 4 partials per batch element.

v2 layout/schedule notes:
  - Phase-interleaved emission: QKV(half1) and the output projection are
    woven between attention score/exp/PV steps so the PE (matmul) and
    ACT (exp) engines overlap instead of running in serial phases.
  - Scores S^T[k,q] for the two heads of a pair go to different PE
    row-groups (base partitions 0/64 -> tile_position row tiling) and
    run concurrently; exp covers both heads' tiles in one [128,2048]
    ACTIVATE (amortizes the ~350-cycle ACT instruction overhead).
  - Causal masking adds a NEG triangle only on the 128-wide diagonal
    band via one narrowed identity matmul per (group, head); score and
    PV matmuls are narrowed to live columns.
  - RoPE half-rotation is a within-quadrant partition swap done by DVE
    stream_shuffle; the host permutes the qk feature rows so rotation
    pairs are 16 partitions apart (dot products are permutation
    invariant, v/out projection unaffected).
  - Output is written bf16 (host upcasts and sums partials).
"""

import sys
import numpy as np

if '/opt/trn_rl_repo' not in sys.path:
    sys.path.insert(0, '/opt/trn_rl_repo')

import ml_dtypes

B, T, C, H, D = 2, 2048, 1024, 16, 64
HPC = 4            # heads per core
NCORES = 8
NEG = -1.0e9
BF = ml_dtypes.bfloat16

QT = T // 2        # tokens per half

# rope feature permutation: within each head's 64 rows put rotation
# partners 16 apart so the swap stays inside a 32-partition quadrant
P64 = np.concatenate([np.arange(0, 16), np.arange(32, 48),
                      np.arange(16, 32), np.arange(48, 64)])
P128 = np.concatenate([P64, 64 + P64])
SHUF_MASK = [(i + 16) % 32 for i in range(32)]

_cache = {}


def _build():
    import concourse.mybir as mybir
    from concourse import bacc
    import concourse.tile as tile

    F32 = mybir.dt.float32
    FR = mybir.dt.float32r
    B16 = mybir.dt.bfloat16
    AF = mybir.ActivationFunctionType

    nc = bacc.Bacc("TRN2", debug=False, enable_asserts=True)
    xT = nc.dram_tensor("xT", [C, T], B16, kind="ExternalInput").ap()
    wqk = nc.dram_tensor("wqk", [C, 512], B16, kind="ExternalInput").ap()
    wv = nc.dram_tensor("wv", [C, 256], B16, kind="ExternalInput").ap()
    wo = nc.dram_tensor("wo", [256, 1024], B16, kind="ExternalInput").ap()
    cosR = nc.dram_tensor("cosR", [128, T], B16, kind="ExternalInput").ap()
    sinS = nc.dram_tensor("sinS", [128, T], B16, kind="ExternalInput").ap()
    ident = nc.dram_tensor("ident", [128, 128], B16, kind="ExternalInput").ap()
    triM = nc.dram_tensor("triM", [128, 128], B16, kind="ExternalInput").ap()
    onesI = nc.dram_tensor("onesI", [128, 64], FR, kind="ExternalInput").ap()
    out = nc.dram_tensor("out", [T, C], B16, kind="ExternalOutput").ap()

    with tile.TileContext(nc) as tc:
        with tc.tile_pool(name="persist", bufs=1) as pp, \
             tc.tile_pool(name="xqp", bufs=2) as xqp, \
             tc.tile_pool(name="rawp", bufs=4) as rawp, \
             tc.tile_pool(name="swpp", bufs=2) as swpp, \
             tc.tile_pool(name="ptp", bufs=3) as ptp, \
             tc.tile_pool(name="stagep", bufs=8) as stagep, \
             tc.tile_pool(name="nrmp", bufs=8) as nrmp, \
             tc.tile_pool(name="ysbp", bufs=3) as ysbp, \
             tc.tile_pool(name="pST", bufs=1, space="PSUM") as pST, \
             tc.tile_pool(name="pOUT", bufs=2, space="PSUM") as pOUT, \
             tc.tile_pool(name="pMISC", bufs=2, space="PSUM") as pMISC:

            # ---- persistent SBUF tensors ----
            wqk_sb = pp.tile([128, 8 * 512], B16, tag="wqk")
            wv_sb = pp.tile([128, 8 * 256], B16, tag="wv")
            wo_sb = pp.tile([128, 2 * 1024], B16, tag="wo")
            cos_sb = pp.tile([128, T], B16, tag="cos")
            sin_sb = pp.tile([128, T], B16, tag="sin")
            id_sb = pp.tile([128, 128], B16, tag="id")
            tri_sb = pp.tile([128, 128], B16, tag="tri")
            ones_sb = pp.tile([128, 64], FR, tag="ones")
            onesb_sb = pp.tile([128, 64], B16, tag="onesb")
            scratch = pp.tile([128, 640], B16, tag="scratch")
            rot = [pp.tile([128, T], B16, tag=f"rot{i}", name=f"rot{i}")
                   for i in range(4)]
            v_sb = pp.tile([128, 16 * HPC * 65], B16, tag="v")
            aou = [pp.tile([128, T], B16, tag=f"aou{i}", name=f"aou{i}")
                   for i in range(2)]

            ST = pST.tile([128, 2048], F32, tag="ST")

            # ---- warmup: keep HAM busy while input DMAs run ----
            nc.vector.memset(scratch[:], 0.0)
            wps = pMISC.tile([128, 512], F32, tag="misc", name="warm")
            for i in range(20):
                nc.tensor.matmul(wps[:], scratch[:, 0:128], scratch[:, 128:640],
                                 start=True, stop=True)

            # ---- input DMA issue (ordered by need, single queue) ----
            xq0 = xqp.tile([128, 8 * QT], B16, tag="xq", name="xq0")
            xq1 = xqp.tile([128, 8 * QT], B16, tag="xq", name="xq1")
            for k in range(4):
                nc.sync.dma_start(wqk_sb[:, k * 512:(k + 1) * 512],
                                  wqk[k * 128:(k + 1) * 128, :])
                nc.sync.dma_start(xq0[:, k * QT:(k + 1) * QT],
                                  xT[k * 128:(k + 1) * 128, 0:QT])
            nc.sync.dma_start(cos_sb[:], cosR[:])
            nc.sync.dma_start(sin_sb[:], sinS[:])
            for k in range(4, 8):
                nc.sync.dma_start(wqk_sb[:, k * 512:(k + 1) * 512],
                                  wqk[k * 128:(k + 1) * 128, :])
                nc.sync.dma_start(xq0[:, k * QT:(k + 1) * QT],
                                  xT[k * 128:(k + 1) * 128, 0:QT])
            for k in range(8):
                nc.sync.dma_start(wv_sb[:, k * 256:(k + 1) * 256],
                                  wv[k * 128:(k + 1) * 128, :])
            nc.sync.dma_start(ones_sb[:], onesI[:])
            nc.sync.dma_start(id_sb[:], ident[:])
            nc.sync.dma_start(tri_sb[:], triM[:])
            for k in range(2):
                nc.sync.dma_start(wo_sb[:, k * 1024:(k + 1) * 1024],
                                  wo[k * 128:(k + 1) * 128, :])
            for k in range(8):
                nc.sync.dma_start(xq1[:, k * QT:(k + 1) * QT],
                                  xT[k * 128:(k + 1) * 128, QT:T])

            # ================= QKV machinery =================
            def rope(m, raw, t0):
                swp = swpp.tile([128, QT], B16, tag="swp")
                nc.vector.stream_shuffle(swp[:], raw[:], SHUF_MASK)
                tmp = swpp.tile([128, QT], B16, tag="tmp")
                nc.vector.tensor_mul(tmp[:], swp[:], sin_sb[:, t0:t0 + QT])
                rt = rot[m]
                nc.vector.tensor_mul(rt[:, t0:t0 + QT], raw[:],
                                     cos_sb[:, t0:t0 + QT])
                nc.vector.tensor_add(rt[:, t0:t0 + QT], rt[:, t0:t0 + QT],
                                     tmp[:])

            def qk_kmajor_ST(mpair, xq, t0, half):
                """phase-A only: k-major over two m chains in the 4 ST banks"""
                for k in range(8):
                    for j, m in enumerate(mpair):
                        for n in range(2):
                            nc.tensor.matmul(
                                ST[:, (2 * j + n) * 512:(2 * j + n + 1) * 512],
                                wqk_sb[:, k * 512 + m * 128:
                                       k * 512 + (m + 1) * 128],
                                xq[:, k * QT + n * 512:k * QT + (n + 1) * 512],
                                start=(k == 0), stop=(k == 7))
                for j, m in enumerate(mpair):
                    raw = rawp.tile([128, QT], B16, tag="raw",
                                    name=f"rawA{half}_{m}")
                    nc.scalar.copy(raw[:], ST[:, j * 1024:(j + 1) * 1024])
                    rope(m, raw, t0)

            def v_chain(mt, xq, qhalf, psv, i, do_copy):
                for k in range(8):
                    nc.tensor.matmul(
                        psv[:, i * 256:(i + 1) * 256],
                        xq[:, k * QT + mt * 128:k * QT + (mt + 1) * 128],
                        wv_sb[:, k * 256:(k + 1) * 256],
                        start=(k == 0), stop=(k == 7))
                if do_copy:
                    kb0 = qhalf * 8 + mt - 1
                    base = kb0 * 4 * 65
                    vsrc = psv[:].rearrange("p (t h d) -> p t h d", t=2, h=4)
                    vdst = v_sb[:, base:base + 2 * 4 * 65].rearrange(
                        "p (t h d) -> p t h d", t=2, d=65)[:, :, :, 0:64]
                    nc.scalar.copy(vdst, vsrc)

            # ================= attention machinery =================
            norm1 = {}

            def norm_part1(qb, pi, h2, out_ps):
                stage = stagep.tile([65, 512], F32, tag="stage",
                                    name=f"stage{qb}_{pi}_{h2}")
                nc.vector.tensor_copy(stage[:], out_ps[:])
                den0 = nrmp.tile([1, 512], F32, tag="den0")
                nc.gpsimd.dma_start(den0[:], stage[64:65, :])
                rr = nrmp.tile([1, 512], F32, tag="rr")
                with nc.allow_low_precision(reason="softmax denominators"):
                    nc.vector.reciprocal_approx_fast(rr[:], den0[:])
                    rrr = nrmp.tile([1, 512], FR, tag="rrr",
                                    name=f"rrr{qb}_{pi}_{h2}")
                    nc.vector.tensor_copy(rrr[:], rr[:])
                norm1[(qb, pi, h2)] = (stage, rrr)

            def norm_part2(qb, pi, h2):
                stage, rrr = norm1.pop((qb, pi, h2))
                bc = pMISC.tile([64, 512], F32, tag="misc",
                                name=f"bc{qb}_{pi}_{h2}")
                nc.tensor.matmul(bc[:], ones_sb[0:1, 0:64], rrr[:],
                                 start=True, stop=True)
                nstage = nrmp.tile([64, 512], B16, tag="nstage")
                nc.vector.tensor_mul(nstage[:], stage[0:64, :], bc[:])
                nc.gpsimd.dma_start(
                    aou[pi][64 * h2:64 * h2 + 64, qb * 512:(qb + 1) * 512],
                    nstage[:])

            def outproj_piece(qt, nh):
                yp = pMISC.tile([128, 512], F32, tag="misc",
                                name=f"yp{qt}_{nh}")
                nc.tensor.matmul(yp[:], aou[0][:, qt * 128:(qt + 1) * 128],
                                 wo_sb[:, nh * 512:(nh + 1) * 512],
                                 start=True, stop=False)
                nc.tensor.matmul(yp[:], aou[1][:, qt * 128:(qt + 1) * 128],
                                 wo_sb[:, 1024 + nh * 512:1024 + (nh + 1) * 512],
                                 start=False, stop=True)
                ysb = ysbp.tile([128, 512], B16, tag="ysb",
                                name=f"ysb{qt}_{nh}")
                nc.vector.tensor_copy(ysb[:], yp[:])
                nc.sync.dma_start(
                    out[qt * 128:(qt + 1) * 128, nh * 512:(nh + 1) * 512],
                    ysb[:])

            # work queue of small PE chunks woven between attention steps
            work = []

            def pull_work(n):
                for _ in range(n):
                    if work:
                        work.pop(0)()

            def attn_round(qb, pi, pulls_per_g):
                Qt, Kt = rot[pi], rot[2 + pi]
                ng = 2 * (qb + 1)
                live = 4 * (qb + 1)
                ops = {h2: pOUT.tile([65, 512], F32, tag="out_ps",
                                     name=f"ops{qb}_{pi}_{h2}")
                       for h2 in range(2)}
                pts = {}
                for g in range(ng + 1):
                    if g < ng:
                        # per-head score group + exp halves: head h2's
                        # next-group scores overlap the other head's exp
                        pt = ptp.tile([128, 2048], B16, tag="pt",
                                      name=f"pt{qb}_{pi}_{g}")
                        for h2 in range(2):
                            for i in range(2):
                                kb = 2 * g + i
                                diag = kb >= 4 * qb
                                dl = (kb - 4 * qb) * 128 if diag else 0
                                nc.tensor.matmul(
                                    ST[:, h2 * 1024 + i * 512 + dl:
                                       h2 * 1024 + (i + 1) * 512],
                                    Kt[64 * h2:64 * h2 + 64,
                                       kb * 128:(kb + 1) * 128],
                                    Qt[64 * h2:64 * h2 + 64,
                                       qb * 512 + dl:(qb + 1) * 512],
                                    start=True, stop=(not diag))
                            for i in range(2):
                                kb = 2 * g + i
                                if kb >= 4 * qb:
                                    # NEG triangle on the 128-wide band
                                    dl = (kb - 4 * qb) * 128
                                    o = h2 * 1024 + i * 512 + dl
                                    nc.tensor.matmul(
                                        ST[:, o:o + 128], id_sb[:], tri_sb[:],
                                        start=False, stop=True)
                            # leading dead columns of this head's group
                            dl0 = (2 * g - 4 * qb) * 128 if 2 * g >= 4 * qb else 0
                            nc.scalar.activation(
                                pt[:, h2 * 1024 + dl0:(h2 + 1) * 1024],
                                ST[:, h2 * 1024 + dl0:(h2 + 1) * 1024],
                                AF.Exp, scale=0.125)
                        pts[g] = pt
                    if g >= 1:
                        pt = pts.pop(g - 1)
                        for h2 in range(2):
                            for i in range(2):
                                kb = 2 * (g - 1) + i
                                diag = kb >= 4 * qb
                                dl = (kb - 4 * qb) * 128 if diag else 0
                                nc.tensor.matmul(
                                    ops[h2][:, dl:512],
                                    v_sb[:, (kb * 4 + 2 * pi + h2) * 65:
                                         (kb * 4 + 2 * pi + h2) * 65 + 65],
                                    pt[:, h2 * 1024 + i * 512 + dl:
                                       h2 * 1024 + (i + 1) * 512],
                                    start=(kb == 0), stop=(kb == live - 1))
                    pull_work(pulls_per_g)
                for h2 in range(2):
                    norm_part1(qb, pi, h2, ops[h2])

            # ================= emission =================
            # ---- phase A: QKV half 0 ----
            qk_kmajor_ST((0, 1), xq0, 0, 0)
            qk_kmajor_ST((2, 3), xq0, 0, 0)
            for vp in range(4):
                psv = pMISC.tile([128, 512], F32, tag="misc", name=f"psvA{vp}")
                v_chain(2 * vp, xq0, 0, psv, 0, False)
                v_chain(2 * vp + 1, xq0, 0, psv, 1, True)

            nc.vector.tensor_copy(onesb_sb[:], ones_sb[:])
            # ones column at col 64 of every v slot
            nc.vector.tensor_copy(
                v_sb.rearrange("p (s d) -> p s d", d=65)[:, :, 64:65],
                onesb_sb[:, 0:64].unsqueeze(2))

            # ---- phase B work: QKV half 1 in small chunks ----
            class Lazy:
                """allocate the chain's psum tiles at weave time"""
                def __init__(self, shapes, names):
                    self.shapes, self.names, self.t = shapes, names, None
                def get(self):
                    if self.t is None:
                        self.t = [pMISC.tile(s, F32, tag="misc", name=n)
                                  for s, n in zip(self.shapes, self.names)]
                    return self.t

            def qk_chunk(m, ks, lz, half_done):
                ps = lz.get()
                for k in ks:
                    for n in range(2):
                        nc.tensor.matmul(
                            ps[n][:],
                            wqk_sb[:, k * 512 + m * 128:
                                   k * 512 + (m + 1) * 128],
                            xq1[:, k * QT + n * 512:k * QT + (n + 1) * 512],
                            start=(k == 0), stop=(k == 7))
                if half_done:
                    raw = rawp.tile([128, QT], B16, tag="raw",
                                    name=f"rawB{m}")
                    nc.scalar.copy(raw[:, 0:512], ps[0][:])
                    nc.scalar.copy(raw[:, 512:1024], ps[1][:])
                    rope(m, raw, QT)

            for m in range(4):
                lz = Lazy([[128, 512]] * 2, [f"psB{m}a", f"psB{m}b"])
                work.append(lambda lz=lz, m=m: qk_chunk(m, range(0, 4), lz, False))
                work.append(lambda lz=lz, m=m: qk_chunk(m, range(4, 8), lz, True))
            for vp in range(4):
                lz = Lazy([[128, 512]], [f"psvB{vp}"])
                work.append(lambda lz=lz, vp=vp:
                            v_chain(2 * vp, xq1, 1, lz.get()[0], 0, False))
                work.append(lambda lz=lz, vp=vp:
                            v_chain(2 * vp + 1, xq1, 1, lz.get()[0], 1, True))

            # rounds: (qb, pair); qb3 before qb2 to shorten the tail
            rounds = [(0, 0), (0, 1), (1, 0), (1, 1),
                      (3, 0), (3, 1), (2, 0), (2, 1)]
            outproj_before = {3: 0, 5: 1, 7: 3}  # round idx -> qb to append
            for ri, (qb, pi) in enumerate(rounds):
                if ri >= 2:
                    pqb, ppi = rounds[ri - 2]
                    for h2 in range(2):
                        norm_part2(pqb, ppi, h2)
                if ri in outproj_before:
                    oqb = outproj_before[ri]
                    for qt in range(4 * oqb, 4 * oqb + 4):
                        for nh in range(2):
                            work.append(lambda qt=qt, nh=nh:
                                        outproj_piece(qt, nh))
                pulls = 2 if ri < 4 else 1
                attn_round(qb, pi, pulls)
            for (pqb, ppi) in rounds[6:]:
                for h2 in range(2):
                    norm_part2(pqb, ppi, h2)
            pull_work(len(work))
            for qt in range(8, 12):
                for nh in range(2):
                    outproj_piece(qt, nh)

    nc.compile()
    return nc


def _core_inputs(x, cos, sin, W_qkv, W_out, core):
    b = core // 4
    hg = core % 4
    heads = list(range(4 * hg, 4 * hg + 4))

    xT = np.ascontiguousarray(x[b].T).astype(BF)
    qrows = np.concatenate([W_qkv[h * 64:(h + 1) * 64] for h in heads], 0)
    krows = np.concatenate([W_qkv[C + h * 64: C + (h + 1) * 64] for h in heads], 0)
    wqk = np.ascontiguousarray(np.concatenate([qrows, krows], 0).T)  # (C, 512)
    # permute rope-pair feature columns within each 128-col m-block
    wqk = wqk.reshape(C, 4, 128)[:, :, P128].reshape(C, 512)
    wqk = np.ascontiguousarray(wqk).astype(BF)
    vrows = np.concatenate([W_qkv[2 * C + h * 64: 2 * C + (h + 1) * 64] for h in heads], 0)
    wv = np.ascontiguousarray(vrows.T).astype(BF)
    cols = np.concatenate([np.arange(h * 64, (h + 1) * 64) for h in heads])
    wo = np.ascontiguousarray(W_out[:, cols].T).astype(BF)

    cT = np.ascontiguousarray(cos.T)      # (32, T)
    sT = np.ascontiguousarray(sin.T)
    cosR = np.tile(cT, (4, 1))[P128].astype(BF)
    sinS = np.concatenate([-sT, sT, -sT, sT], 0)[P128].astype(BF)

    p = np.arange(128)[:, None]
    j = np.arange(128)[None, :]
    triM = np.where(j < p, NEG, 0.0).astype(BF)

    return {
        "xT": xT, "wqk": wqk, "wv": wv, "wo": wo,
        "cosR": cosR, "sinS": sinS,
        "ident": np.eye(128).astype(BF),
        "triM": np.ascontiguousarray(triM),
        "onesI": np.ones((128, 64), dtype=np.float32),
    }


def kernel(x, cos, sin, mask, W_qkv, W_out):
    from concourse import bass_utils

    x = np.asarray(x, dtype=np.float32)
    cos = np.asarray(cos, dtype=np.float32)
    sin = np.asarray(sin, dtype=np.float32)
    W_qkv = np.asarray(W_qkv, dtype=np.float32)
    W_out = np.asarray(W_out, dtype=np.float32)

    if "nc" not in _cache:
        _cache["nc"] = _build()
    nc = _cache["nc"]

    in_maps = [_core_inputs(x, cos, sin, W_qkv, W_out, c) for c in range(NCORES)]
    res = bass_utils.run_bass_kernel_spmd(nc, in_maps, core_ids=list(range(NCORES)))

    y = np.zeros((B, T, C), dtype=np.float32)
    for c in range(NCORES):
        y[c // 4] += np.asarray(res.results[c]["out"], dtype=np.float32)
    return y
